# revision 33
# baseline (speedup 1.0000x reference)
"""Trainium2 Bass kernel for nn_EnhancedSemGCN.

Data-parallel over batch: B=16 samples, 8 NeuronCores, 2 samples per core.
Params replicated, per-sample attention/adjacency/GCN computed locally.

Reference computation per sample (S=512, D=768, H=8, DK=96):
  q/k projections -> 8-head scores + relative-position bias (from head-mean
  query, clipped distance embedding) + token mask -> softmax -> head-mean
  adjacency with forced self-loops + row mask -> 2-layer dense GCN
  (adj @ x @ W, relu, /denom) -> weighted fusion -> Wf -> global avg/max pool
  -> 2-layer gate -> outputs = x + gate*gc.  Returns (outputs, adj).

Layout strategy on device (matmuls run at full PE rate via float32r/bf16):
  - qT/kT computed transposed (d on partitions) directly: lhsT=W, rhs=X.T
  - scores_h = qT_h.T @ kT_h with the (rel+mask) bias preloaded into PSUM
    via an identity matmul; exp on ACT with fused row-sum accumulation;
    per-head 1/(8Z) diagonal rescale accumulated into PSUM -> adjacency
  - rel-position band realized by writing padded per-row projections to a
    DRAM scratch and reading back with a diagonal access pattern
  - GCN alternates layouts naturally: axT = out_nat.T-contract, out_nat =
    axT.T-contract; biases folded in as K=1 ones-row matmuls
  - global pooling via PE transpose + ACT accumulate / DVE max-reduce
"""

import sys

if "/opt/trn_rl_repo" not in sys.path:
    sys.path.insert(0, "/opt/trn_rl_repo")

import ml_dtypes
import numpy as np

import concourse.bass as bass
import concourse.mybir as mybir
import concourse.tile as tile
from concourse import bacc
from concourse.bass_utils import run_bass_kernel_spmd

f32 = mybir.dt.float32
f32r = mybir.dt.float32r
bf16 = mybir.dt.bfloat16
AF = mybir.ActivationFunctionType
OP = mybir.AluOpType
AX = mybir.AxisListType

B, S, D, H = 16, 512, 768, 8
DK = D // H          # 96
MAXREL = 128
NCORES = 8
BL = B // NCORES     # 2 samples per core
NT = S // 128        # 4 row tiles
ND = D // 128        # 6 feature tiles
NS = 384             # N-slice for D-wide matmul outputs
PADW = 1024          # padded rel row width: 383 | 257 | 384


def _r(ap):
    return ap.bitcast(f32r)


def _bcast_rows(ap_1d, parts):
    """DRAM (n,) AP -> (parts, n) partition-broadcast AP for DMA."""
    return bass.AP(tensor=ap_1d.tensor, offset=ap_1d.offset,
                   ap=[[0, parts]] + list(ap_1d.ap))


def build_nc(r_imm, swap_feats, use_bias=True, num_devices=NCORES):
    """Emit the per-core program. r_imm/swap_feats encode softmax(scale_w):
    combined = w_big * (f_big + r_imm * f_small); w_big is folded into Wf
    on the host."""
    nc = bacc.Bacc("TRN2", target_bir_lowering=False, debug=False,
                   num_devices=num_devices)

    dt_in = lambda n, shp: nc.dram_tensor(n, shp, f32, kind="ExternalInput")
    x_d = nc.dram_tensor("x", [BL, S, D], bf16, kind="ExternalInput")
    mb_d = dt_in("mb", [BL, S])            # 0 / -1e9 key-mask bias
    rm_d = dt_in("rm", [BL, 128, NT])      # row mask tiles in (p, t)
    eye_d = dt_in("eye", [NT, 128, S])     # row blocks of eye(S)
    wq_d = nc.dram_tensor("wq", [D, D], bf16, kind="ExternalInput")             # pre-scaled by 1/sqrt(dk)
    bqh_d = dt_in("bqh", [DK, H])          # pre-scaled, per-head columns
    wk_d = nc.dram_tensor("wk", [D, D], bf16, kind="ExternalInput")
    wqm_d = nc.dram_tensor("wqm", [D, DK], bf16, kind="ExternalInput")
    bqm_d = dt_in("bqm", [DK, 1])
    bkh_d = dt_in("bkh", [DK, H])
    reltp_d = nc.dram_tensor("reltp", [DK, PADW], bf16, kind="ExternalInput")   # rel_emb.T with clip padding
    w0_d = nc.dram_tensor("w0", [D, D], bf16, kind="ExternalInput")
    b0_d = dt_in("b0", [1, D])
    w1_d = nc.dram_tensor("w1", [D, D], bf16, kind="ExternalInput")
    b1_d = dt_in("b1", [1, D])
    wf_d = nc.dram_tensor("wf", [D, D], bf16, kind="ExternalInput")             # pre-scaled by w_big
    bf_d = dt_in("bf", [1, D])
    wfc_d = nc.dram_tensor("wfc", [2 * D, D], bf16, kind="ExternalInput")
    bfc_d = dt_in("bfc", [1, D])
    wg_d = nc.dram_tensor("wg", [D, D], bf16, kind="ExternalInput")
    bg_d = dt_in("bg", [1, D])
    out_d = nc.dram_tensor("out", [BL, S, D], f32, kind="ExternalOutput")
    adj_d = nc.dram_tensor("adj", [BL, S, S], f32, kind="ExternalOutput")

    def load_w6(pool, name, dram):
        t = pool.tile([128, dram.shape[0] // 128, D], bf16, tag=name)
        nc.sync.dma_start(
            out=t[:], in_=dram[:].rearrange("(t p) d -> p t d", p=128))
        return t

    with tile.TileContext(nc) as tc:
        with (
            tc.tile_pool(name="single", bufs=1) as single,
            tc.tile_pool(name="persist", bufs=2) as persist,
            tc.tile_pool(name="smalls", bufs=8) as smalls,
        ):
            # ---- constants ----
            I_sb = single.tile([128, 128], f32, tag="I")
            nc.gpsimd.memset(I_sb[:], 1.0)
            nc.gpsimd.affine_select(
                out=I_sb[:], in_=I_sb[:], compare_op=OP.is_equal, fill=0.0,
                base=0, pattern=[[-1, 128]], channel_multiplier=1)
            Io8 = single.tile([128, 128], f32, tag="Io8")
            nc.vector.tensor_scalar_mul(Io8[:], I_sb[:], 1.0 / H)
            I_bf = single.tile([128, 128], bf16, tag="Ibf")
            nc.vector.tensor_copy(I_bf[:], I_sb[:])
            Io8_bf = single.tile([128, 128], bf16, tag="Io8bf")
            nc.vector.tensor_copy(Io8_bf[:], Io8[:])
            ones_sb = single.tile([1, D], f32, tag="ones")
            nc.vector.memset(ones_sb[:], 1.0)
            zb2 = single.tile([BL, 1], f32, tag="zb2")
            nc.vector.memset(zb2[:], 0.0)
            cat_sb = single.tile([128, 2 * ND, BL], bf16, tag="cat")

            xnat = [None] * BL
            adjT = [None] * BL
            rden = [None] * BL
            comb = [None] * BL
            xx = [None] * BL

            # ================= ATTENTION =================
            with (
                tc.tile_pool(name="aw", bufs=1) as aw,
                tc.tile_pool(name="adr", bufs=2, space="DRAM") as adr,
                tc.tile_pool(name="aps", bufs=2, space="PSUM") as aps,
                tc.tile_pool(name="aps3", bufs=4, space="PSUM") as aps3,
            ):
                wq_sb = load_w6(aw, "wq", wq_d)
                wk_sb = load_w6(aw, "wk", wk_d)
                eye_sb = aw.tile([128, NT, S], bf16, tag="eye")
                nc.gpsimd.dma_start(
                    out=eye_sb[:], in_=eye_d[:].rearrange("t p j -> p t j"))
                om_sb = aw.tile([128, NT, S], bf16, tag="om")
                nc.vector.tensor_scalar(om_sb[:], eye_sb[:], -1.0, 1.0,
                                        OP.mult, OP.add)
                reltp_sb = aw.tile([DK, PADW], bf16, tag="reltp")
                nc.sync.dma_start(out=reltp_sb[:], in_=reltp_d[:])
                bqh_sb = aw.tile([DK, H], f32, tag="bqh")
                nc.sync.dma_start(out=bqh_sb[:], in_=bqh_d[:])
                bkh_sb = aw.tile([DK, H], f32, tag="bkh")
                nc.sync.dma_start(out=bkh_sb[:], in_=bkh_d[:])
                wqm_sb = aw.tile([128, ND, DK], bf16, tag="wqm")
                nc.sync.dma_start(
                    out=wqm_sb[:],
                    in_=wqm_d[:].rearrange("(t p) d -> p t d", p=128))
                bqm_sb = aw.tile([DK, 1], f32, tag="bqm")
                nc.sync.dma_start(out=bqm_sb[:], in_=bqm_d[:])

                for s in range(BL):
                    with (
                        tc.tile_pool(name="as1", bufs=1) as as1,
                        tc.tile_pool(name="as2", bufs=2) as as2,
                        tc.tile_pool(name="as8", bufs=12) as as8,
                    ):
                        mb_sb = as1.tile([128, S], bf16, tag="mb")
                        nc.gpsimd.dma_start(out=mb_sb[:],
                                           in_=_bcast_rows(mb_d[s], 128))
                        rm_sb = as1.tile([128, NT], f32, tag="rm")
                        nc.sync.dma_start(out=rm_sb[:], in_=rm_d[s])
                        xn = persist.tile([128, NT, D], bf16, tag="xnat")
                        nc.sync.dma_start(
                            out=xn[:],
                            in_=x_d[s].rearrange("(t p) d -> p t d", p=128))
                        xnat[s] = xn

                        # X.T via PE transpose
                        XT = as1.tile([128, ND, S], bf16, tag="XT")
                        for dt in range(ND):
                            pxt = aps.tile([128, S], bf16, tag="qk")
                            for it in range(NT):
                                nc.tensor.transpose(
                                    pxt[:, 128 * it:128 * (it + 1)],
                                    xn[:, it, 128 * dt:128 * (dt + 1)],
                                    I_bf[:])
                            nc.vector.tensor_copy(XT[:, dt, :], pxt[:])

                        # head-mean query directly from host-folded
                        # Wq_mean: lets the rel/premask chain overlap the
                        # per-head projections below
                        pqm = aps.tile([DK, S], f32, tag="qk")
                        for kt in range(ND):
                            nc.tensor.matmul(pqm[:], wqm_sb[:, kt, :],
                                             XT[:, kt, :],
                                             start=(kt == 0),
                                             stop=(kt == ND - 1))
                        qm = as1.tile([DK, S], bf16, tag="qm")
                        nc.scalar.activation(qm[:], pqm[:], AF.Identity,
                                             bias=bqm_sb[:])

                        # rel-position scores: projS = qm.T @ reltp (padded),
                        # bounce through DRAM, read diagonally, add key mask
                        prem = as1.tile([128, NT, S], bf16, tag="prem")
                        for t in range(NT):
                            dsc = adr.tile([128, PADW], f32, tag="bscr")
                            pad_sb = as1.tile([128, PADW], f32, tag="pad")
                            for hf in range(2):
                                pp = aps3.tile([128, 512], f32, tag="sc")
                                nc.tensor.matmul(
                                    pp[:],
                                    qm[:, 128 * t:128 * (t + 1)],
                                    reltp_sb[:, 512 * hf:512 * (hf + 1)],
                                    start=True, stop=True)
                                nc.vector.tensor_copy(
                                    pad_sb[:, 512 * hf:512 * (hf + 1)], pp[:])
                            nc.sync.dma_start(out=dsc[:], in_=pad_sb[:])
                            full = dsc[:]
                            diag_ap = bass.AP(
                                tensor=full.tensor,
                                offset=full.offset + (PADW // 2 - 1) - 128 * t,
                                ap=[[PADW - 1, 128], [1, S]])
                            band = as2.tile([128, S], f32, tag="band")
                            nc.sync.dma_start(out=band[:], in_=diag_ap)
                            nc.vector.tensor_add(prem[:, t, :], band[:], mb_sb[:])

                        # per-head qT / kT (dk on partitions), bf16
                        qT = as1.tile([DK, H, S], bf16, tag="qT")
                        kT = as1.tile([DK, H, S], bf16, tag="kT")
                        for w_sb, bh_sb, dst in ((wq_sb, bqh_sb, qT),
                                                 (wk_sb, bkh_sb, kT)):
                            for h in range(H):
                                pq = aps.tile([DK, S], f32, tag="qk")
                                for kt in range(ND):
                                    nc.tensor.matmul(
                                        pq[:],
                                        w_sb[:, kt, DK * h:DK * (h + 1)],
                                        XT[:, kt, :],
                                        start=(kt == 0), stop=(kt == ND - 1))
                                nc.vector.tensor_scalar_add(
                                    dst[:, h, :], pq[:], bh_sb[:, h:h + 1])

                        # scores -> softmax -> head-mean adjacency
                        # waves per t: all scores MMs, all exps, all rescale
                        # diags, then the accumulate MMs (keeps the PE queue
                        # free of the exp->recip->diag latency)
                        adjf = as1.tile([128, NT, S], f32, tag="adjf")
                        rd = persist.tile([128, NT], f32, tag="rden")

                        waves = []

                        def emit_scores(t):
                            pscs = []
                            for h in range(H):
                                psc = aps3.tile([128, S], f32, tag="sc")
                                nc.tensor.matmul(psc[:], I_bf[:],
                                                 prem[:, t, :],
                                                 start=True, stop=False)
                                nc.tensor.matmul(
                                    psc[:],
                                    qT[:, h, 128 * t:128 * (t + 1)],
                                    kT[:, h, :],
                                    start=False, stop=True)
                                pscs.append(psc)
                            es, dgs = [], []
                            for h in range(H):
                                e_sb = as8.tile([128, S], bf16, tag="e")
                                zs = smalls.tile([128, 1], f32, tag="Z")
                                nc.scalar.activation(e_sb[:], pscs[h][:],
                                                     AF.Exp, accum_out=zs[:])
                                es.append(e_sb)
                                rcp = smalls.tile([128, 1], f32, tag="rcp")
                                nc.vector.reciprocal(rcp[:], zs[:])
                                dg = as8.tile([128, 128], bf16, tag="dg")
                                nc.vector.tensor_scalar_mul(dg[:], Io8[:], rcp[:])
                                dgs.append(dg)
                            return es, dgs

                        def emit_accs(t, es, dgs):
                            padj = aps.tile([128, S], f32, tag="adj")
                            for h in range(H):
                                nc.tensor.matmul(padj[:], dgs[h][:], es[h][:],
                                                 start=(h == 0),
                                                 stop=(h == H - 1),
                                                 skip_group_check=True)
                            # adj = rowmask*(p_mean*(1-eye) + eye); denom=rs+1
                            tmp = as2.tile([128, S], f32, tag="tmpa")
                            nc.vector.scalar_tensor_tensor(
                                tmp[:], padj[:], rm_sb[:, t:t + 1],
                                om_sb[:, t, :], op0=OP.mult, op1=OP.mult)
                            eyem = as2.tile([128, S], bf16, tag="eyem")
                            nc.vector.tensor_scalar_mul(
                                eyem[:], eye_sb[:, t, :], rm_sb[:, t:t + 1])
                            rs = smalls.tile([128, 1], f32, tag="rs")
                            nc.vector.scalar_tensor_tensor(
                                adjf[:, t, :], tmp[:], 1.0, eyem[:],
                                op0=OP.mult, op1=OP.add, accum_out=rs[:])
                            den = smalls.tile([128, 1], f32, tag="den")
                            nc.vector.tensor_scalar_add(den[:], rs[:], 1.0)
                            nc.vector.reciprocal(rd[:, t:t + 1], den[:])
                            nc.sync.dma_start(
                                out=adj_d[s].rearrange(
                                    "(t p) j -> p t j", p=128)[:, t, :],
                                in_=adjf[:, t, :])

                        for t in range(NT):
                            waves.append(emit_scores(t))
                            if t >= 1:
                                emit_accs(t - 1, *waves[t - 1])
                        emit_accs(NT - 1, *waves[NT - 1])
                        rden[s] = rd

                        # adjT for the GCN contraction
                        aT = persist.tile([128, NT, S], bf16, tag="adjT")
                        for jt in range(NT):
                            pat = aps.tile([128, S], f32, tag="qk")
                            for it in range(NT):
                                nc.tensor.transpose(
                                    pat[:, 128 * it:128 * (it + 1)],
                                    adjf[:, it, 128 * jt:128 * (jt + 1)],
                                    I_sb[:])
                            nc.vector.tensor_copy(aT[:, jt, :], pat[:])
                        adjT[s] = aT

            # ============ GCN + FUSION + POOLING ============
            with (
                tc.tile_pool(name="gw", bufs=1) as gw,
                tc.tile_pool(name="gt1", bufs=1) as gt1,
                tc.tile_pool(name="adr2", bufs=1, space="DRAM") as adr2,
                tc.tile_pool(name="gps", bufs=2, space="PSUM") as gps,
                tc.tile_pool(name="gps3", bufs=4, space="PSUM") as gps3,
            ):
                w0_sb = load_w6(gw, "w0", w0_d)
                w1_sb = load_w6(gw, "w1", w1_d)
                wf_sb = load_w6(gw, "wf", wf_d)
                wfc_sb = load_w6(gw, "wfc", wfc_d)
                wg_sb = load_w6(gw, "wg", wg_d)
                bfcr = gw.tile([1, D], f32r, tag="bfc")
                nc.sync.dma_start(out=bfcr[:], in_=bfc_d[:].bitcast(f32r))
                bgr = gw.tile([1, D], f32r, tag="bg")
                nc.sync.dma_start(out=bgr[:], in_=bg_d[:].bitcast(f32r))
                b0r = gw.tile([1, D], f32r, tag="b0")
                nc.sync.dma_start(out=b0r[:], in_=b0_d[:].bitcast(f32r))
                b1r = gw.tile([1, D], f32r, tag="b1")
                nc.sync.dma_start(out=b1r[:], in_=b1_d[:].bitcast(f32r))
                bfr = gw.tile([1, D], f32r, tag="bf")
                nc.sync.dma_start(out=bfr[:], in_=bf_d[:].bitcast(f32r))

                curs = [xnat[si] for si in range(BL)]
                feats_sl = [[None, None] for _ in range(BL)]
                for l in range(2):
                    wl, blr = ((w0_sb, b0r), (w1_sb, b1r))[l]
                    for s in range(BL):
                        axT = gt1.tile([128, ND, S], bf16, tag=f"axT{s}")
                        for dt in range(ND):
                            pax = gps.tile([128, S], f32, tag="ax")
                            for jt in range(NT):
                                nc.tensor.matmul(
                                    pax[:],
                                    curs[s][:, jt, 128 * dt:128 * (dt + 1)],
                                    adjT[s][:, jt, :],
                                    start=(jt == 0), stop=(jt == NT - 1))
                            nc.scalar.copy(axT[:, dt, :], pax[:])
                        new = gt1.tile([128, NT, D], bf16, tag=f"f{l}s{s}")
                        for it in range(NT):
                            for ns in range(2):
                                po = gps3.tile([128, NS], f32, tag="o")
                                for kt in range(ND):
                                    nc.tensor.matmul(
                                        po[:],
                                        axT[:, kt, 128 * it:128 * (it + 1)],
                                        wl[:, kt, NS * ns:NS * (ns + 1)],
                                        start=(kt == 0),
                                        stop=(not use_bias and kt == ND - 1))
                                if use_bias:
                                    nc.tensor.matmul(
                                        po[:], _r(ones_sb[0:1, 0:128]),
                                        blr[0:1, NS * ns:NS * (ns + 1)],
                                        start=False, stop=True)
                                # relu((ax@W + b) / denom)
                                nc.vector.tensor_scalar(
                                    new[:, it, NS * ns:NS * (ns + 1)], po[:],
                                    rden[s][:, it:it + 1], 0.0, OP.mult, OP.max)
                        feats_sl[s][l] = new
                        curs[s] = new

                for s in range(BL):
                    feats = feats_sl[s]
                    # fusion: comb = f_big + r_imm * f_small (w_big in Wf)
                    f1, f2 = feats
                    big, small = (f2, f1) if swap_feats else (f1, f2)
                    cb = gt1.tile([128, NT, D], bf16, tag="comb")
                    for it in range(NT):
                        nc.vector.scalar_tensor_tensor(
                            cb[:, it, :], small[:, it, :], r_imm, big[:, it, :],
                            op0=OP.mult, op1=OP.add)
                    combT = gt1.tile([128, ND, S], bf16, tag="combT")
                    for dt in range(ND):
                        ptr = gps.tile([128, S], bf16, tag="tr")
                        for it in range(NT):
                            nc.tensor.transpose(
                                ptr[:, 128 * it:128 * (it + 1)],
                                cb[:, it, 128 * dt:128 * (dt + 1)], I_bf[:])
                        nc.scalar.copy(combT[:, dt, :], ptr[:])
                    xs = persist.tile([128, NT, D], f32, tag="xx")
                    for it in range(NT):
                        for ns in range(2):
                            px = gps3.tile([128, NS], f32, tag="o")
                            for kt in range(ND):
                                nc.tensor.matmul(
                                    px[:],
                                    combT[:, kt, 128 * it:128 * (it + 1)],
                                    wf_sb[:, kt, NS * ns:NS * (ns + 1)],
                                    start=(kt == 0),
                                    stop=(not use_bias and kt == ND - 1))
                            if use_bias:
                                nc.tensor.matmul(
                                    px[:], _r(ones_sb[0:1, 0:128]),
                                    bfr[0:1, NS * ns:NS * (ns + 1)],
                                    start=False, stop=True)
                            nc.scalar.copy(xs[:, it, NS * ns:NS * (ns + 1)],
                                           px[:])
                    xx[s] = xs
                    # global avg/max pool over the sequence (via transpose)
                    for dt in range(ND):
                        pxt2 = gps.tile([128, S], f32, tag="tr")
                        for it in range(NT):
                            nc.tensor.transpose(
                                pxt2[:, 128 * it:128 * (it + 1)],
                                xs[:, it, 128 * dt:128 * (dt + 1)], I_sb[:])
                        scrap = gt1.tile([128, S], f32, tag="scrap")
                        gsum = smalls.tile([128, 1], f32, tag="gsum")
                        nc.scalar.activation(scrap[:], pxt2[:], AF.Copy,
                                             accum_out=gsum[:])
                        gmax = smalls.tile([128, 1], f32, tag="gmax")
                        nc.vector.reduce_max(out=gmax[:], in_=pxt2[:], axis=AX.X)
                        nc.vector.tensor_scalar_mul(cat_sb[:, dt, s:s + 1],
                                                    gsum[:], 1.0 / S)
                        nc.vector.tensor_copy(cat_sb[:, ND + dt, s:s + 1],
                                              gmax[:])

                    # ---- gating for this sample (overlaps next fusion) ----
                    gc_sb = gt1.tile([1, D], f32, tag="gc")
                    for ns in range(2):
                        pgc = gps3.tile([1, NS], f32, tag="o")
                        for j in range(2 * ND):
                            nc.tensor.matmul(
                                pgc[:], cat_sb[:, j, s:s + 1],
                                wfc_sb[:, j, NS * ns:NS * (ns + 1)],
                                start=(j == 0),
                                stop=(not use_bias and j == 2 * ND - 1))
                        if use_bias:
                            nc.tensor.matmul(pgc[:], _r(ones_sb[0:1, 0:1]),
                                             bfcr[0:1, NS * ns:NS * (ns + 1)],
                                             start=False, stop=True)
                        nc.scalar.copy(gc_sb[:, NS * ns:NS * (ns + 1)], pgc[:])
                    gcT = gt1.tile([128, ND, 1], bf16, tag="gcT")
                    for dt in range(ND):
                        pgt = gps.tile([128, 1], f32, tag="tr")
                        nc.tensor.transpose(
                            pgt[:], gc_sb[:, 128 * dt:128 * (dt + 1)],
                            I_sb[0:1, 0:1])
                        nc.vector.tensor_copy(gcT[:, dt, :], pgt[:])
                    gate_sb = gt1.tile([1, D], f32, tag="gate")
                    for ns in range(2):
                        pg = gps.tile([1, NS], f32, tag="ax")
                        for dt in range(ND):
                            nc.tensor.matmul(
                                pg[:], gcT[:, dt, :],
                                wg_sb[:, dt, NS * ns:NS * (ns + 1)],
                                start=(dt == 0),
                                stop=(not use_bias and dt == ND - 1))
                        if use_bias:
                            nc.tensor.matmul(pg[:], _r(ones_sb[0:1, 0:1]),
                                             bgr[0:1, NS * ns:NS * (ns + 1)],
                                             start=False, stop=True)
                        nc.scalar.activation(gate_sb[:, NS * ns:NS * (ns + 1)],
                                             pg[:], AF.Sigmoid,
                                             bias=zb2[0:1, :])
                    gg_sb = gt1.tile([1, D], f32, tag="ggs")
                    nc.vector.tensor_mul(gg_sb[:], gate_sb[:], gc_sb[:])
                    ggd = adr2.tile([1, D], f32, tag="ggd")
                    nc.gpsimd.dma_start(out=ggd[:], in_=gg_sb[:])
                    ggb = gt1.tile([128, D], f32, tag="ggb")
                    nc.gpsimd.dma_start(out=ggb[:], in_=_bcast_rows(ggd[0], 128))
                    for it in range(NT):
                        nc.vector.tensor_add(xx[s][:, it, :], xx[s][:, it, :],
                                             ggb[:])
                        nc.gpsimd.dma_start(
                            out=out_d[s].rearrange(
                                "(t p) d -> p t d", p=128)[:, it, :],
                            in_=xx[s][:, it, :])

    nc.compile()
    return nc


def host_prep(inputs):
    """Host-side input prep: sharding + small derived tensors."""
    g = lambda n: np.asarray(inputs[n], dtype=np.float32)
    x = g("inputs")
    tok = np.asarray(inputs["tok"])
    scale = 1.0 / np.sqrt(np.float32(DK))

    wq = g("Wq") * scale
    bqh = (np.asarray(inputs["bq"], np.float32) * scale).reshape(H, DK).T.copy()
    wk = g("Wk")
    bkh = g("bk").reshape(H, DK).T.copy()

    re = g("rel_emb")  # (257, 96)
    reltp = np.concatenate(
        [np.tile(re[0:1], (PADW // 2 - MAXREL - 1, 1)), re,
         np.tile(re[-1:], (PADW // 2 - MAXREL, 1))], axis=0).T.copy()  # (96,1024)

    sw = g("scale_w").astype(np.float64)
    e = np.exp(sw - sw.max())
    w = (e / e.sum()).astype(np.float32)
    swap = bool(w[1] > w[0])
    w_big = w[1] if swap else w[0]
    w_small = w[0] if swap else w[1]
    r_imm = float(w_small / w_big)
    wf = g("Wf") * w_big

    mb = np.where(tok != 0, np.float32(0.0), np.float32(-1e9)).astype(np.float32)
    rmask = (tok != 0).astype(np.float32).reshape(B, NT, 128).transpose(0, 2, 1).copy()
    eye = np.eye(S, dtype=np.float32).reshape(NT, 128, S)

    wqm = wq.reshape(D, H, DK).mean(axis=1)
    bqm = (np.asarray(inputs["bq"], np.float32) * scale).reshape(
        H, DK).mean(axis=0).reshape(DK, 1)
    b16 = lambda a: np.asarray(a, dtype=ml_dtypes.bfloat16)
    shared = {
        "eye": eye, "wq": b16(wq), "bqh": bqh, "wk": b16(g("Wk")),
        "bkh": bkh, "reltp": b16(reltp), "wqm": b16(wqm), "bqm": bqm,
        "w0": b16(g("W0")), "b0": g("b0").reshape(1, D), "w1": b16(g("W1")),
        "b1": g("b1").reshape(1, D), "wf": b16(wf),
        "bf": g("bf").reshape(1, D),
        "wfc": b16(g("Wfc")), "bfc": g("bfc").reshape(1, D),
        "wg": b16(g("Wg")), "bg": g("bg").reshape(1, D),
    }
    use_bias = any(
        np.abs(np.asarray(inputs[n], np.float32)).max() > 0
        for n in ("b0", "b1", "bf", "bfc", "bg"))
    per_core = []
    for c in range(NCORES):
        sl = slice(BL * c, BL * (c + 1))
        m = dict(shared)
        m["x"] = b16(np.ascontiguousarray(x[sl]))
        m["mb"] = np.ascontiguousarray(mb[sl])
        m["rm"] = np.ascontiguousarray(rmask[sl])
        per_core.append(m)
    return per_core, r_imm, swap, use_bias


_NC_CACHE = {}


def kernel(**inputs):
    per_core, r_imm, swap, use_bias = host_prep(inputs)
    key = (round(r_imm, 9), swap, use_bias)
    if key not in _NC_CACHE:
        _NC_CACHE[key] = build_nc(r_imm, swap, use_bias)
    nc = _NC_CACHE[key]
    res = run_bass_kernel_spmd(nc, per_core, list(range(NCORES)))
    outputs = np.concatenate([res.results[c]["out"] for c in range(NCORES)], axis=0)
    adj = np.concatenate([res.results[c]["adj"] for c in range(NCORES)], axis=0)
    return outputs.astype(np.float32), adj.astype(np.float32)


# revision 34
# speedup vs baseline: 1.0254x; 1.0254x over previous
"""Trainium2 Bass kernel for nn_EnhancedSemGCN.

Data-parallel over batch: B=16 samples, 8 NeuronCores, 2 samples per core.
Params replicated, per-sample attention/adjacency/GCN computed locally.

Reference computation per sample (S=512, D=768, H=8, DK=96):
  q/k projections -> 8-head scores + relative-position bias (from head-mean
  query, clipped distance embedding) + token mask -> softmax -> head-mean
  adjacency with forced self-loops + row mask -> 2-layer dense GCN
  (adj @ x @ W, relu, /denom) -> weighted fusion -> Wf -> global avg/max pool
  -> 2-layer gate -> outputs = x + gate*gc.  Returns (outputs, adj).

Layout strategy on device (matmuls run at full PE rate via float32r/bf16):
  - qT/kT computed transposed (d on partitions) directly: lhsT=W, rhs=X.T
  - scores_h = qT_h.T @ kT_h with the (rel+mask) bias preloaded into PSUM
    via an identity matmul; exp on ACT with fused row-sum accumulation;
    per-head 1/(8Z) diagonal rescale accumulated into PSUM -> adjacency
  - rel-position band realized by writing padded per-row projections to a
    DRAM scratch and reading back with a diagonal access pattern
  - GCN alternates layouts naturally: axT = out_nat.T-contract, out_nat =
    axT.T-contract; biases folded in as K=1 ones-row matmuls
  - global pooling via PE transpose + ACT accumulate / DVE max-reduce
"""

import sys

if "/opt/trn_rl_repo" not in sys.path:
    sys.path.insert(0, "/opt/trn_rl_repo")

import ml_dtypes
import numpy as np

import concourse.bass as bass
import concourse.mybir as mybir
import concourse.tile as tile
from concourse import bacc
from concourse.bass_utils import run_bass_kernel_spmd

f32 = mybir.dt.float32
f32r = mybir.dt.float32r
bf16 = mybir.dt.bfloat16
AF = mybir.ActivationFunctionType
OP = mybir.AluOpType
AX = mybir.AxisListType

B, S, D, H = 16, 512, 768, 8
DK = D // H          # 96
MAXREL = 128
NCORES = 8
BL = B // NCORES     # 2 samples per core
NT = S // 128        # 4 row tiles
ND = D // 128        # 6 feature tiles
NS = 384             # N-slice for D-wide matmul outputs
PADW = 1024          # padded rel row width: 383 | 257 | 384


def _r(ap):
    return ap.bitcast(f32r)


def _bcast_rows(ap_1d, parts):
    """DRAM (n,) AP -> (parts, n) partition-broadcast AP for DMA."""
    return bass.AP(tensor=ap_1d.tensor, offset=ap_1d.offset,
                   ap=[[0, parts]] + list(ap_1d.ap))


def build_nc(r_imm, swap_feats, use_bias=True, num_devices=NCORES):
    """Emit the per-core program. r_imm/swap_feats encode softmax(scale_w):
    combined = w_big * (f_big + r_imm * f_small); w_big is folded into Wf
    on the host."""
    nc = bacc.Bacc("TRN2", target_bir_lowering=False, debug=False,
                   num_devices=num_devices)

    dt_in = lambda n, shp: nc.dram_tensor(n, shp, f32, kind="ExternalInput")
    x_d = nc.dram_tensor("x", [BL, S, D], bf16, kind="ExternalInput")
    mb_d = dt_in("mb", [BL, S])            # 0 / -1e9 key-mask bias
    rm_d = dt_in("rm", [BL, 128, NT])      # row mask tiles in (p, t)
    eye_d = dt_in("eye", [NT, 128, S])     # row blocks of eye(S)
    wq_d = nc.dram_tensor("wq", [D, D], bf16, kind="ExternalInput")             # pre-scaled by 1/sqrt(dk)
    bqh_d = dt_in("bqh", [DK, H])          # pre-scaled, per-head columns
    wk_d = nc.dram_tensor("wk", [D, D], bf16, kind="ExternalInput")
    wqm_d = nc.dram_tensor("wqm", [D, DK], bf16, kind="ExternalInput")
    bqm_d = dt_in("bqm", [DK, 1])
    bkh_d = dt_in("bkh", [DK, H])
    reltp_d = nc.dram_tensor("reltp", [DK, PADW], bf16, kind="ExternalInput")   # rel_emb.T with clip padding
    w0_d = nc.dram_tensor("w0", [D, D], bf16, kind="ExternalInput")
    b0_d = dt_in("b0", [1, D])
    w1_d = nc.dram_tensor("w1", [D, D], bf16, kind="ExternalInput")
    b1_d = dt_in("b1", [1, D])
    wf_d = nc.dram_tensor("wf", [D, D], bf16, kind="ExternalInput")             # pre-scaled by w_big
    bf_d = dt_in("bf", [1, D])
    wfc_d = nc.dram_tensor("wfc", [2 * D, D], bf16, kind="ExternalInput")
    bfc_d = dt_in("bfc", [1, D])
    wg_d = nc.dram_tensor("wg", [D, D], bf16, kind="ExternalInput")
    bg_d = dt_in("bg", [1, D])
    out_d = nc.dram_tensor("out", [BL, S, D], f32, kind="ExternalOutput")
    adj_d = nc.dram_tensor("adj", [BL, S, S], f32, kind="ExternalOutput")

    def load_w6(pool, name, dram):
        t = pool.tile([128, dram.shape[0] // 128, D], bf16, tag=name)
        nc.sync.dma_start(
            out=t[:], in_=dram[:].rearrange("(t p) d -> p t d", p=128))
        return t

    with tile.TileContext(nc) as tc:
        with (
            tc.tile_pool(name="single", bufs=1) as single,
            tc.tile_pool(name="persist", bufs=2) as persist,
            tc.tile_pool(name="smalls", bufs=16) as smalls,
        ):
            # ---- constants ----
            I_sb = single.tile([128, 128], f32, tag="I")
            nc.gpsimd.memset(I_sb[:], 1.0)
            nc.gpsimd.affine_select(
                out=I_sb[:], in_=I_sb[:], compare_op=OP.is_equal, fill=0.0,
                base=0, pattern=[[-1, 128]], channel_multiplier=1)
            Io8 = single.tile([128, 128], f32, tag="Io8")
            nc.vector.tensor_scalar_mul(Io8[:], I_sb[:], 1.0 / H)
            I_bf = single.tile([128, 128], bf16, tag="Ibf")
            nc.vector.tensor_copy(I_bf[:], I_sb[:])
            Io8_bf = single.tile([128, 128], bf16, tag="Io8bf")
            nc.vector.tensor_copy(Io8_bf[:], Io8[:])
            ones_sb = single.tile([1, D], f32, tag="ones")
            nc.vector.memset(ones_sb[:], 1.0)
            zb2 = single.tile([BL, 1], f32, tag="zb2")
            nc.vector.memset(zb2[:], 0.0)
            cat_sb = single.tile([128, 2 * ND, BL], bf16, tag="cat")

            xnat = [None] * BL
            adjT = [None] * BL
            rden = [None] * BL
            comb = [None] * BL
            xx = [None] * BL

            # ================= ATTENTION =================
            with (
                tc.tile_pool(name="aw", bufs=1) as aw,
                tc.tile_pool(name="adr", bufs=2, space="DRAM") as adr,
                tc.tile_pool(name="aps", bufs=2, space="PSUM") as aps,
                tc.tile_pool(name="aps3", bufs=4, space="PSUM") as aps3,
            ):
                wq_sb = load_w6(aw, "wq", wq_d)
                wk_sb = load_w6(aw, "wk", wk_d)
                eye_sb = aw.tile([128, NT, S], bf16, tag="eye")
                nc.gpsimd.dma_start(
                    out=eye_sb[:], in_=eye_d[:].rearrange("t p j -> p t j"))
                om_sb = aw.tile([128, NT, S], bf16, tag="om")
                nc.vector.tensor_scalar(om_sb[:], eye_sb[:], -1.0, 1.0,
                                        OP.mult, OP.add)
                reltp_sb = aw.tile([DK, PADW], bf16, tag="reltp")
                nc.sync.dma_start(out=reltp_sb[:], in_=reltp_d[:])
                bqh_sb = aw.tile([DK, H], f32, tag="bqh")
                nc.sync.dma_start(out=bqh_sb[:], in_=bqh_d[:])
                bkh_sb = aw.tile([DK, H], f32, tag="bkh")
                nc.sync.dma_start(out=bkh_sb[:], in_=bkh_d[:])
                wqm_sb = aw.tile([128, ND, DK], bf16, tag="wqm")
                nc.sync.dma_start(
                    out=wqm_sb[:],
                    in_=wqm_d[:].rearrange("(t p) d -> p t d", p=128))
                bqm_sb = aw.tile([DK, 1], f32, tag="bqm")
                nc.sync.dma_start(out=bqm_sb[:], in_=bqm_d[:])

                for s in range(BL):
                    with (
                        tc.tile_pool(name="as1", bufs=1) as as1,
                        tc.tile_pool(name="as2", bufs=2) as as2,
                        tc.tile_pool(name="as8", bufs=12) as as8,
                    ):
                        mb_sb = as1.tile([128, S], bf16, tag="mb")
                        nc.gpsimd.dma_start(out=mb_sb[:],
                                           in_=_bcast_rows(mb_d[s], 128))
                        rm_sb = as1.tile([128, NT], f32, tag="rm")
                        nc.sync.dma_start(out=rm_sb[:], in_=rm_d[s])
                        xn = persist.tile([128, NT, D], bf16, tag="xnat")
                        nc.sync.dma_start(
                            out=xn[:],
                            in_=x_d[s].rearrange("(t p) d -> p t d", p=128))
                        xnat[s] = xn

                        # X.T via PE transpose
                        XT = as1.tile([128, ND, S], bf16, tag="XT")
                        for dt in range(ND):
                            pxt = aps.tile([128, S], bf16, tag="qk")
                            for it in range(NT):
                                nc.tensor.transpose(
                                    pxt[:, 128 * it:128 * (it + 1)],
                                    xn[:, it, 128 * dt:128 * (dt + 1)],
                                    I_bf[:])
                            nc.vector.tensor_copy(XT[:, dt, :], pxt[:])

                        # head-mean query directly from host-folded
                        # Wq_mean: lets the rel/premask chain overlap the
                        # per-head projections below
                        pqm = aps.tile([DK, S], f32, tag="qk")
                        for kt in range(ND):
                            nc.tensor.matmul(pqm[:], wqm_sb[:, kt, :],
                                             XT[:, kt, :],
                                             start=(kt == 0),
                                             stop=(kt == ND - 1))
                        qm = as1.tile([DK, S], bf16, tag="qm")
                        nc.scalar.activation(qm[:], pqm[:], AF.Identity,
                                             bias=bqm_sb[:])

                        # rel-position scores: projS = qm.T @ reltp (padded),
                        # bounce through DRAM, read diagonally, add key mask
                        prem = as1.tile([128, NT, S], bf16, tag="prem")
                        for t in range(NT):
                            dsc = adr.tile([128, PADW], f32, tag="bscr")
                            pad_sb = as1.tile([128, PADW], f32, tag="pad")
                            for hf in range(2):
                                pp = aps3.tile([128, 512], f32, tag="sc")
                                nc.tensor.matmul(
                                    pp[:],
                                    qm[:, 128 * t:128 * (t + 1)],
                                    reltp_sb[:, 512 * hf:512 * (hf + 1)],
                                    start=True, stop=True)
                                nc.vector.tensor_copy(
                                    pad_sb[:, 512 * hf:512 * (hf + 1)], pp[:])
                            nc.sync.dma_start(out=dsc[:], in_=pad_sb[:])
                            full = dsc[:]
                            diag_ap = bass.AP(
                                tensor=full.tensor,
                                offset=full.offset + (PADW // 2 - 1) - 128 * t,
                                ap=[[PADW - 1, 128], [1, S]])
                            band = as2.tile([128, S], f32, tag="band")
                            nc.sync.dma_start(out=band[:], in_=diag_ap)
                            nc.vector.tensor_add(prem[:, t, :], band[:], mb_sb[:])

                        # per-head qT / kT (dk on partitions), bf16
                        qT = as1.tile([DK, H, S], bf16, tag="qT")
                        kT = as1.tile([DK, H, S], bf16, tag="kT")
                        for w_sb, bh_sb, dst in ((wq_sb, bqh_sb, qT),
                                                 (wk_sb, bkh_sb, kT)):
                            for h in range(H):
                                pq = aps.tile([DK, S], f32, tag="qk")
                                for kt in range(ND):
                                    nc.tensor.matmul(
                                        pq[:],
                                        w_sb[:, kt, DK * h:DK * (h + 1)],
                                        XT[:, kt, :],
                                        start=(kt == 0), stop=(kt == ND - 1))
                                nc.vector.tensor_scalar_add(
                                    dst[:, h, :], pq[:], bh_sb[:, h:h + 1])

                        # scores -> softmax -> head-mean adjacency
                        # waves per t: all scores MMs, all exps, all rescale
                        # diags, then the accumulate MMs (keeps the PE queue
                        # free of the exp->recip->diag latency)
                        adjf = as1.tile([128, NT, S], f32, tag="adjf")
                        rd = persist.tile([128, NT], f32, tag="rden")

                        waves = []

                        def emit_scores(t):
                            pscs = []
                            for h in range(H):
                                psc = aps3.tile([128, S], f32, tag="sc")
                                nc.tensor.matmul(psc[:], I_bf[:],
                                                 prem[:, t, :],
                                                 start=True, stop=False)
                                nc.tensor.matmul(
                                    psc[:],
                                    qT[:, h, 128 * t:128 * (t + 1)],
                                    kT[:, h, :],
                                    start=False, stop=True)
                                pscs.append(psc)
                            es, dgs = [], []
                            for h in range(H):
                                e_sb = as8.tile([128, S], bf16, tag="e")
                                zs = smalls.tile([128, 1], f32, tag="Z")
                                nc.scalar.activation(e_sb[:], pscs[h][:],
                                                     AF.Exp, accum_out=zs[:])
                                es.append(e_sb)
                                rcp = smalls.tile([128, 1], f32, tag="rcp")
                                nc.vector.reciprocal(rcp[:], zs[:])
                                dg = as8.tile([128, 128], bf16, tag="dg")
                                nc.vector.tensor_scalar_mul(dg[:], Io8[:], rcp[:])
                                dgs.append(dg)
                            return es, dgs

                        def emit_accs(t, es, dgs):
                            padj = aps.tile([128, S], f32, tag="adj")
                            for h in range(H):
                                nc.tensor.matmul(padj[:], dgs[h][:], es[h][:],
                                                 start=(h == 0),
                                                 stop=(h == H - 1),
                                                 skip_group_check=True)
                            # adj = rowmask*(p_mean*(1-eye) + eye); denom=rs+1
                            tmp = as2.tile([128, S], f32, tag="tmpa")
                            nc.vector.scalar_tensor_tensor(
                                tmp[:], padj[:], rm_sb[:, t:t + 1],
                                om_sb[:, t, :], op0=OP.mult, op1=OP.mult)
                            eyem = as2.tile([128, S], bf16, tag="eyem")
                            nc.vector.tensor_scalar_mul(
                                eyem[:], eye_sb[:, t, :], rm_sb[:, t:t + 1])
                            rs = smalls.tile([128, 1], f32, tag="rs")
                            nc.vector.scalar_tensor_tensor(
                                adjf[:, t, :], tmp[:], 1.0, eyem[:],
                                op0=OP.mult, op1=OP.add, accum_out=rs[:])
                            den = smalls.tile([128, 1], f32, tag="den")
                            nc.vector.tensor_scalar_add(den[:], rs[:], 1.0)
                            nc.vector.reciprocal(rd[:, t:t + 1], den[:])
                            nc.sync.dma_start(
                                out=adj_d[s].rearrange(
                                    "(t p) j -> p t j", p=128)[:, t, :],
                                in_=adjf[:, t, :])

                        for t in range(NT):
                            waves.append(emit_scores(t))
                            if t >= 1:
                                emit_accs(t - 1, *waves[t - 1])
                        emit_accs(NT - 1, *waves[NT - 1])
                        rden[s] = rd

                        # adjT for the GCN contraction
                        aT = persist.tile([128, NT, S], bf16, tag="adjT")
                        for jt in range(NT):
                            pat = aps.tile([128, S], f32, tag="qk")
                            for it in range(NT):
                                nc.tensor.transpose(
                                    pat[:, 128 * it:128 * (it + 1)],
                                    adjf[:, it, 128 * jt:128 * (jt + 1)],
                                    I_sb[:])
                            nc.vector.tensor_copy(aT[:, jt, :], pat[:])
                        adjT[s] = aT

            # ============ GCN + FUSION + POOLING ============
            with (
                tc.tile_pool(name="gw", bufs=1) as gw,
                tc.tile_pool(name="gt1", bufs=1) as gt1,
                tc.tile_pool(name="adr2", bufs=1, space="DRAM") as adr2,
                tc.tile_pool(name="gps", bufs=2, space="PSUM") as gps,
                tc.tile_pool(name="gps3", bufs=4, space="PSUM") as gps3,
            ):
                w0_sb = load_w6(gw, "w0", w0_d)
                w1_sb = load_w6(gw, "w1", w1_d)
                wf_sb = load_w6(gw, "wf", wf_d)
                wfc_sb = load_w6(gw, "wfc", wfc_d)
                wg_sb = load_w6(gw, "wg", wg_d)
                bfcr = gw.tile([1, D], f32r, tag="bfc")
                nc.sync.dma_start(out=bfcr[:], in_=bfc_d[:].bitcast(f32r))
                bgr = gw.tile([1, D], f32r, tag="bg")
                nc.sync.dma_start(out=bgr[:], in_=bg_d[:].bitcast(f32r))
                b0r = gw.tile([1, D], f32r, tag="b0")
                nc.sync.dma_start(out=b0r[:], in_=b0_d[:].bitcast(f32r))
                b1r = gw.tile([1, D], f32r, tag="b1")
                nc.sync.dma_start(out=b1r[:], in_=b1_d[:].bitcast(f32r))
                bfr = gw.tile([1, D], f32r, tag="bf")
                nc.sync.dma_start(out=bfr[:], in_=bf_d[:].bitcast(f32r))

                curs = [xnat[si] for si in range(BL)]
                feats_sl = [[None, None] for _ in range(BL)]
                for l in range(2):
                    wl, blr = ((w0_sb, b0r), (w1_sb, b1r))[l]
                    for s in range(BL):
                        axT = gt1.tile([128, ND, S], bf16, tag=f"axT{s}")
                        for dt in range(ND):
                            pax = gps.tile([128, S], f32, tag="ax")
                            for jt in range(NT):
                                nc.tensor.matmul(
                                    pax[:],
                                    curs[s][:, jt, 128 * dt:128 * (dt + 1)],
                                    adjT[s][:, jt, :],
                                    start=(jt == 0), stop=(jt == NT - 1))
                            nc.scalar.copy(axT[:, dt, :], pax[:])
                        new = gt1.tile([128, NT, D], bf16, tag=f"f{l}s{s}")
                        for it in range(NT):
                            for ns in range(2):
                                po = gps3.tile([128, NS], f32, tag="o")
                                for kt in range(ND):
                                    nc.tensor.matmul(
                                        po[:],
                                        axT[:, kt, 128 * it:128 * (it + 1)],
                                        wl[:, kt, NS * ns:NS * (ns + 1)],
                                        start=(kt == 0),
                                        stop=(not use_bias and kt == ND - 1))
                                if use_bias:
                                    nc.tensor.matmul(
                                        po[:], _r(ones_sb[0:1, 0:128]),
                                        blr[0:1, NS * ns:NS * (ns + 1)],
                                        start=False, stop=True)
                                # relu((ax@W + b) / denom)
                                nc.vector.tensor_scalar(
                                    new[:, it, NS * ns:NS * (ns + 1)], po[:],
                                    rden[s][:, it:it + 1], 0.0, OP.mult, OP.max)
                        feats_sl[s][l] = new
                        curs[s] = new

                for s in range(BL):
                    feats = feats_sl[s]
                    # fusion: comb = f_big + r_imm * f_small (w_big in Wf)
                    f1, f2 = feats
                    big, small = (f2, f1) if swap_feats else (f1, f2)
                    cb = gt1.tile([128, NT, D], bf16, tag="comb")
                    for it in range(NT):
                        nc.vector.scalar_tensor_tensor(
                            cb[:, it, :], small[:, it, :], r_imm, big[:, it, :],
                            op0=OP.mult, op1=OP.add)
                    combT = gt1.tile([128, ND, S], bf16, tag="combT")
                    for dt in range(ND):
                        ptr = gps.tile([128, S], bf16, tag="tr")
                        for it in range(NT):
                            nc.tensor.transpose(
                                ptr[:, 128 * it:128 * (it + 1)],
                                cb[:, it, 128 * dt:128 * (dt + 1)], I_bf[:])
                        nc.scalar.copy(combT[:, dt, :], ptr[:])
                    xs = persist.tile([128, NT, D], f32, tag="xx")
                    for it in range(NT):
                        for ns in range(2):
                            px = gps3.tile([128, NS], f32, tag="o")
                            for kt in range(ND):
                                nc.tensor.matmul(
                                    px[:],
                                    combT[:, kt, 128 * it:128 * (it + 1)],
                                    wf_sb[:, kt, NS * ns:NS * (ns + 1)],
                                    start=(kt == 0),
                                    stop=(not use_bias and kt == ND - 1))
                            if use_bias:
                                nc.tensor.matmul(
                                    px[:], _r(ones_sb[0:1, 0:128]),
                                    bfr[0:1, NS * ns:NS * (ns + 1)],
                                    start=False, stop=True)
                            nc.scalar.copy(xs[:, it, NS * ns:NS * (ns + 1)],
                                           px[:])
                    xx[s] = xs
                    # global avg/max pool over the sequence (via transpose)
                    for dt in range(ND):
                        pxt2 = gps.tile([128, S], f32, tag="tr")
                        for it in range(NT):
                            nc.tensor.transpose(
                                pxt2[:, 128 * it:128 * (it + 1)],
                                xs[:, it, 128 * dt:128 * (dt + 1)], I_sb[:])
                        scrap = gt1.tile([128, S], f32, tag="scrap")
                        gsum = smalls.tile([128, 1], f32, tag="gsum")
                        nc.scalar.activation(scrap[:], pxt2[:], AF.Copy,
                                             accum_out=gsum[:])
                        gmax = smalls.tile([128, 1], f32, tag="gmax")
                        nc.vector.reduce_max(out=gmax[:], in_=pxt2[:], axis=AX.X)
                        nc.vector.tensor_scalar_mul(cat_sb[:, dt, s:s + 1],
                                                    gsum[:], 1.0 / S)
                        nc.vector.tensor_copy(cat_sb[:, ND + dt, s:s + 1],
                                              gmax[:])

                    # ---- gating for this sample (overlaps next fusion) ----
                    gc_sb = gt1.tile([1, D], f32, tag="gc")
                    for ns in range(2):
                        pgc = gps3.tile([1, NS], f32, tag="o")
                        for j in range(2 * ND):
                            nc.tensor.matmul(
                                pgc[:], cat_sb[:, j, s:s + 1],
                                wfc_sb[:, j, NS * ns:NS * (ns + 1)],
                                start=(j == 0),
                                stop=(not use_bias and j == 2 * ND - 1))
                        if use_bias:
                            nc.tensor.matmul(pgc[:], _r(ones_sb[0:1, 0:1]),
                                             bfcr[0:1, NS * ns:NS * (ns + 1)],
                                             start=False, stop=True)
                        nc.scalar.copy(gc_sb[:, NS * ns:NS * (ns + 1)], pgc[:])
                    gcT = gt1.tile([128, ND, 1], bf16, tag="gcT")
                    for dt in range(ND):
                        pgt = gps.tile([128, 1], f32, tag="tr")
                        nc.tensor.transpose(
                            pgt[:], gc_sb[:, 128 * dt:128 * (dt + 1)],
                            I_sb[0:1, 0:1])
                        nc.vector.tensor_copy(gcT[:, dt, :], pgt[:])
                    gate_sb = gt1.tile([1, D], f32, tag="gate")
                    for ns in range(2):
                        pg = gps.tile([1, NS], f32, tag="ax")
                        for dt in range(ND):
                            nc.tensor.matmul(
                                pg[:], gcT[:, dt, :],
                                wg_sb[:, dt, NS * ns:NS * (ns + 1)],
                                start=(dt == 0),
                                stop=(not use_bias and dt == ND - 1))
                        if use_bias:
                            nc.tensor.matmul(pg[:], _r(ones_sb[0:1, 0:1]),
                                             bgr[0:1, NS * ns:NS * (ns + 1)],
                                             start=False, stop=True)
                        nc.scalar.activation(gate_sb[:, NS * ns:NS * (ns + 1)],
                                             pg[:], AF.Sigmoid,
                                             bias=zb2[0:1, :])
                    gg_sb = gt1.tile([1, D], f32, tag="ggs")
                    nc.vector.tensor_mul(gg_sb[:], gate_sb[:], gc_sb[:])
                    ggd = adr2.tile([1, D], f32, tag="ggd")
                    nc.gpsimd.dma_start(out=ggd[:], in_=gg_sb[:])
                    ggb = gt1.tile([128, D], f32, tag="ggb")
                    nc.gpsimd.dma_start(out=ggb[:], in_=_bcast_rows(ggd[0], 128))
                    for it in range(NT):
                        nc.vector.tensor_add(xx[s][:, it, :], xx[s][:, it, :],
                                             ggb[:])
                        nc.gpsimd.dma_start(
                            out=out_d[s].rearrange(
                                "(t p) d -> p t d", p=128)[:, it, :],
                            in_=xx[s][:, it, :])

    nc.compile()
    return nc


def host_prep(inputs):
    """Host-side input prep: sharding + small derived tensors."""
    g = lambda n: np.asarray(inputs[n], dtype=np.float32)
    x = g("inputs")
    tok = np.asarray(inputs["tok"])
    scale = 1.0 / np.sqrt(np.float32(DK))

    wq = g("Wq") * scale
    bqh = (np.asarray(inputs["bq"], np.float32) * scale).reshape(H, DK).T.copy()
    wk = g("Wk")
    bkh = g("bk").reshape(H, DK).T.copy()

    re = g("rel_emb")  # (257, 96)
    reltp = np.concatenate(
        [np.tile(re[0:1], (PADW // 2 - MAXREL - 1, 1)), re,
         np.tile(re[-1:], (PADW // 2 - MAXREL, 1))], axis=0).T.copy()  # (96,1024)

    sw = g("scale_w").astype(np.float64)
    e = np.exp(sw - sw.max())
    w = (e / e.sum()).astype(np.float32)
    swap = bool(w[1] > w[0])
    w_big = w[1] if swap else w[0]
    w_small = w[0] if swap else w[1]
    r_imm = float(w_small / w_big)
    wf = g("Wf") * w_big

    mb = np.where(tok != 0, np.float32(0.0), np.float32(-1e9)).astype(np.float32)
    rmask = (tok != 0).astype(np.float32).reshape(B, NT, 128).transpose(0, 2, 1).copy()
    eye = np.eye(S, dtype=np.float32).reshape(NT, 128, S)

    wqm = wq.reshape(D, H, DK).mean(axis=1)
    bqm = (np.asarray(inputs["bq"], np.float32) * scale).reshape(
        H, DK).mean(axis=0).reshape(DK, 1)
    b16 = lambda a: np.asarray(a, dtype=ml_dtypes.bfloat16)
    shared = {
        "eye": eye, "wq": b16(wq), "bqh": bqh, "wk": b16(g("Wk")),
        "bkh": bkh, "reltp": b16(reltp), "wqm": b16(wqm), "bqm": bqm,
        "w0": b16(g("W0")), "b0": g("b0").reshape(1, D), "w1": b16(g("W1")),
        "b1": g("b1").reshape(1, D), "wf": b16(wf),
        "bf": g("bf").reshape(1, D),
        "wfc": b16(g("Wfc")), "bfc": g("bfc").reshape(1, D),
        "wg": b16(g("Wg")), "bg": g("bg").reshape(1, D),
    }
    use_bias = any(
        np.abs(np.asarray(inputs[n], np.float32)).max() > 0
        for n in ("b0", "b1", "bf", "bfc", "bg"))
    per_core = []
    for c in range(NCORES):
        sl = slice(BL * c, BL * (c + 1))
        m = dict(shared)
        m["x"] = b16(np.ascontiguousarray(x[sl]))
        m["mb"] = np.ascontiguousarray(mb[sl])
        m["rm"] = np.ascontiguousarray(rmask[sl])
        per_core.append(m)
    return per_core, r_imm, swap, use_bias


_NC_CACHE = {}


def kernel(**inputs):
    per_core, r_imm, swap, use_bias = host_prep(inputs)
    key = (round(r_imm, 9), swap, use_bias)
    if key not in _NC_CACHE:
        _NC_CACHE[key] = build_nc(r_imm, swap, use_bias)
    nc = _NC_CACHE[key]
    res = run_bass_kernel_spmd(nc, per_core, list(range(NCORES)))
    outputs = np.concatenate([res.results[c]["out"] for c in range(NCORES)], axis=0)
    adj = np.concatenate([res.results[c]["adj"] for c in range(NCORES)], axis=0)
    return outputs.astype(np.float32), adj.astype(np.float32)
